# revision 5
# baseline (speedup 1.0000x reference)
"""Trainium2 Bass kernel for nn_BatchSamplingLoss.

Computes, for each of B=32 pose hypotheses, the masked mean over N=200000
points of || bilinear_sample(img, project(R_b @ (xyz - t_b))) - rgb ||.

Strategy:
  - Shard the B=32 poses across the 8 NeuronCores (4 poses per core),
    replicating the point cloud and the image-derived lookup table.
  - Host-side prep builds a (1025*2050, 12) float32 "pair table": entry
    (r, s) holds the 2x2 bilinear patch [topL, botL, topR, botR] of the
    zero-padded image for y0 = r-1, x0 = s-1.  Zero borders make all
    out-of-bounds taps exact zeros, so no edge-case weight fixups are
    needed on device.
  - On device, per pose, per tile of points: rotate/translate, convert to
    equirectangular pixel coords via Arctan on the scalar engine, derive
    the table index + 4 bilinear weights, gather one 48B record per point
    with gpsimd indirect DMA (128 records per call), blend, and accumulate
    masked distance and mask-count partial sums.
  - The per-partition partial sums [128, 8] per core are returned to the
    host, which reduces across partitions/cores and forms the losses.
"""
import sys

sys.path.insert(0, "/opt/trn_rl_repo")

import numpy as np

import concourse.bass as bass
import concourse.tile as tile
from concourse import bacc, mybir
from concourse.bass_utils import run_bass_kernel_spmd

PI = np.pi
B, N, H, W = 32, 200000, 1024, 2048
NCORES = 8
BPC = B // NCORES            # poses per core
P = 128                      # partitions
L = 1564                     # point columns per partition (128*1564 = 200192 slots)
NCHUNK = 4                   # chunks per pose
T = L // NCHUNK              # 391 columns per chunk
TR, TC = H + 1, W + 2        # pair-table rows/cols (1025 x 2050)
NRECD = 12                   # floats per table record
KY = H / PI                  # 1024/pi
F32 = mybir.dt.float32
I32 = mybir.dt.int32

_CACHED = {}


def _build_bass():
    nc = bacc.Bacc("TRN2", target_bir_lowering=False, debug=False,
                   num_devices=NCORES)
    tbl = nc.dram_tensor("tbl", [TR * TC, NRECD], F32, kind="ExternalInput")
    # point planes: X, Y, Z, then rgb interleaved (t, c), then valid plane
    xyzp = nc.dram_tensor("xyzp", [P, 3 * L], F32, kind="ExternalInput")
    rgbp = nc.dram_tensor("rgbp", [P, 3 * L], F32, kind="ExternalInput")
    vldp = nc.dram_tensor("vldp", [P, L], F32, kind="ExternalInput")
    # per-pose params, replicated across partitions: 6 cols per pose
    # (yaw, pitch, roll, t0, t1, t2) x BPC poses
    pose = nc.dram_tensor("pose", [P, 6 * BPC], F32, kind="ExternalInput")
    out = nc.dram_tensor("out", [P, 2 * BPC], F32, kind="ExternalOutput")

    AL = mybir.AluOpType
    AF = mybir.ActivationFunctionType

    with tile.TileContext(nc) as tc:
        with tc.tile_pool(name="res", bufs=1) as res, \
             tc.tile_pool(name="wk", bufs=2) as wk, \
             tc.tile_pool(name="gp", bufs=2) as gp, \
             tc.tile_pool(name="bl", bufs=2) as bl:

            # ---- resident loads ----
            xyz_t = res.tile([P, 3 * L], F32)
            nc.sync.dma_start(out=xyz_t[:], in_=xyzp[:, :])
            rgb_t = res.tile([P, 3 * L], F32)
            nc.sync.dma_start(out=rgb_t[:], in_=rgbp[:, :])
            vld_t = res.tile([P, L], F32)
            nc.sync.dma_start(out=vld_t[:], in_=vldp[:, :])
            pose_t = res.tile([P, 6 * BPC], F32)
            nc.sync.dma_start(out=pose_t[:], in_=pose[:, :])

            X = xyz_t[:, 0 * L:1 * L]
            Y = xyz_t[:, 1 * L:2 * L]
            Z = xyz_t[:, 2 * L:3 * L]

            # ---- per-pose scalar prep (all BPC poses at once on [P, BPC]) ----
            yaw = pose_t[:, 0 * BPC:1 * BPC]
            pit = pose_t[:, 1 * BPC:2 * BPC]
            rol = pose_t[:, 2 * BPC:3 * BPC]
            t0 = pose_t[:, 3 * BPC:4 * BPC]
            t1 = pose_t[:, 4 * BPC:5 * BPC]
            t2 = pose_t[:, 5 * BPC:6 * BPC]

            halfpi = res.tile([P, 1], F32)
            nc.vector.memset(halfpi[:], PI / 2)
            trig = res.tile([P, 6 * BPC], F32)
            cy, sy = trig[:, 0:BPC], trig[:, BPC:2 * BPC]
            cp, sp = trig[:, 2 * BPC:3 * BPC], trig[:, 3 * BPC:4 * BPC]
            cr, sr = trig[:, 4 * BPC:5 * BPC], trig[:, 5 * BPC:6 * BPC]
            nc.scalar.activation(cy, yaw, AF.Sin, bias=halfpi[:])
            nc.scalar.activation(sy, yaw, AF.Sin)
            nc.scalar.activation(cp, pit, AF.Sin, bias=halfpi[:])
            nc.scalar.activation(sp, pit, AF.Sin)
            nc.scalar.activation(cr, rol, AF.Sin, bias=halfpi[:])
            nc.scalar.activation(sr, rol, AF.Sin)

            # rotation entries (Rm = RZ @ RY @ RX)
            R = res.tile([P, 9 * BPC], F32)

            def rsl(i):
                return R[:, i * BPC:(i + 1) * BPC]

            tmp = res.tile([P, 2 * BPC], F32)
            spsr, spcr = tmp[:, 0:BPC], tmp[:, BPC:2 * BPC]
            tt = nc.vector.tensor_tensor
            tt(out=spsr, in0=sp, in1=sr, op=AL.mult)
            tt(out=spcr, in0=sp, in1=cr, op=AL.mult)
            t3 = res.tile([P, BPC], F32)
            # R00 = cy*cp ; R10 = sy*cp ; R20 = -sp
            tt(out=rsl(0), in0=cy, in1=cp, op=AL.mult)
            tt(out=rsl(3), in0=sy, in1=cp, op=AL.mult)
            nc.vector.tensor_scalar(out=rsl(6), in0=sp, scalar1=-1.0, scalar2=None,
                                    op0=AL.mult)
            # R01 = cy*spsr - sy*cr
            tt(out=rsl(1), in0=cy, in1=spsr, op=AL.mult)
            tt(out=t3[:], in0=sy, in1=cr, op=AL.mult)
            tt(out=rsl(1), in0=rsl(1), in1=t3[:], op=AL.subtract)
            # R02 = cy*spcr + sy*sr
            tt(out=rsl(2), in0=cy, in1=spcr, op=AL.mult)
            tt(out=t3[:], in0=sy, in1=sr, op=AL.mult)
            tt(out=rsl(2), in0=rsl(2), in1=t3[:], op=AL.add)
            # R11 = sy*spsr + cy*cr
            tt(out=rsl(4), in0=sy, in1=spsr, op=AL.mult)
            tt(out=t3[:], in0=cy, in1=cr, op=AL.mult)
            tt(out=rsl(4), in0=rsl(4), in1=t3[:], op=AL.add)
            # R12 = sy*spcr - cy*sr
            tt(out=rsl(5), in0=sy, in1=spcr, op=AL.mult)
            tt(out=t3[:], in0=cy, in1=sr, op=AL.mult)
            tt(out=rsl(5), in0=rsl(5), in1=t3[:], op=AL.subtract)
            # R21 = cp*sr ; R22 = cp*cr
            tt(out=rsl(7), in0=cp, in1=sr, op=AL.mult)
            tt(out=rsl(8), in0=cp, in1=cr, op=AL.mult)

            # c_i = -(R[i,0]*t0 + R[i,1]*t1 + R[i,2]*t2)
            C = res.tile([P, 3 * BPC], F32)
            for i in range(3):
                ci = C[:, i * BPC:(i + 1) * BPC]
                tt(out=ci, in0=rsl(3 * i), in1=t0, op=AL.mult)
                tt(out=t3[:], in0=rsl(3 * i + 1), in1=t1, op=AL.mult)
                tt(out=ci, in0=ci, in1=t3[:], op=AL.add)
                tt(out=t3[:], in0=rsl(3 * i + 2), in1=t2, op=AL.mult)
                tt(out=ci, in0=ci, in1=t3[:], op=AL.add)
                nc.vector.tensor_scalar(out=ci, in0=ci, scalar1=-1.0, scalar2=None,
                                        op0=AL.mult)

            # ---- accumulators ----
            acc = res.tile([P, 2 * BPC], F32)
            nc.vector.memset(acc[:], 0.0)

            ts = nc.vector.tensor_scalar
            stt = nc.vector.scalar_tensor_tensor

            for b in range(BPC):
                def S(i):  # rotation scalar APs for this pose
                    return R[:, i * BPC + b:i * BPC + b + 1]

                def Cs(i):
                    return C[:, i * BPC + b:i * BPC + b + 1]

                for ch in range(NCHUNK):
                    sl = slice(ch * T, (ch + 1) * T)
                    sl3 = slice(ch * 3 * T, (ch + 1) * 3 * T)
                    w = wk.tile([P, 12 * T], F32, tag="w")

                    def ws(i, n=1):
                        return w[:, i * T:(i + n) * T]

                    xr, yr, zr = ws(0), ws(1), ws(2)
                    # rotated coords: xr = R0.(p) + c0 etc.
                    ts(out=xr, in0=X[:, sl], scalar1=S(0), scalar2=Cs(0),
                       op0=AL.mult, op1=AL.add)
                    stt(out=xr, in0=Y[:, sl], scalar=S(1), in1=xr,
                        op0=AL.mult, op1=AL.add)
                    stt(out=xr, in0=Z[:, sl], scalar=S(2), in1=xr,
                        op0=AL.mult, op1=AL.add)
                    ts(out=yr, in0=X[:, sl], scalar1=S(3), scalar2=Cs(1),
                       op0=AL.mult, op1=AL.add)
                    stt(out=yr, in0=Y[:, sl], scalar=S(4), in1=yr,
                        op0=AL.mult, op1=AL.add)
                    stt(out=yr, in0=Z[:, sl], scalar=S(5), in1=yr,
                        op0=AL.mult, op1=AL.add)
                    ts(out=zr, in0=X[:, sl], scalar1=S(6), scalar2=Cs(2),
                       op0=AL.mult, op1=AL.add)
                    stt(out=zr, in0=Y[:, sl], scalar=S(7), in1=zr,
                        op0=AL.mult, op1=AL.add)
                    stt(out=zr, in0=Z[:, sl], scalar=S(8), in1=zr,
                        op0=AL.mult, op1=AL.add)

                    # ---- theta -> ys = y_pix + 1 in [0.5, 1024.5] ----
                    sq, q = ws(3), ws(4)
                    nc.scalar.activation(sq, xr, AF.Square)
                    nc.scalar.activation(q, yr, AF.Square)
                    tt(out=sq, in0=sq, in1=q, op=AL.add)
                    nc.scalar.activation(sq, sq, AF.Sqrt)          # rxy
                    rz = ws(5)
                    nc.vector.reciprocal(out=rz, in_=zr)
                    tt(out=q, in0=sq, in1=rz, op=AL.mult)          # rxy/z
                    ts(out=q, in0=q, scalar1=-1e18, scalar2=1e18,
                       op0=AL.max, op1=AL.min)
                    atn = ws(3)
                    nc.scalar.activation(atn, q, AF.Arctan)
                    ys = ws(4)
                    # A = [z<0] * pi*KY ; B = atn*KY + 0.5 ; ys = A + B
                    ts(out=ys, in0=zr, scalar1=0.0, scalar2=PI * KY,
                       op0=AL.is_lt, op1=AL.mult)
                    nc.scalar.activation(atn, atn, AF.Copy, bias=0.5, scale=KY)
                    tt(out=ys, in0=ys, in1=atn, op=AL.add)

                    # ---- phi -> xs = x_pix + 1 in [0.5, 2048.5) ----
                    rx, q2 = ws(5), ws(6)
                    nc.vector.reciprocal(out=rx, in_=xr)
                    tt(out=q2, in0=yr, in1=rx, op=AL.mult)
                    ts(out=q2, in0=q2, scalar1=-1e18, scalar2=1e18,
                       op0=AL.max, op1=AL.min)
                    atn2 = ws(5)
                    nc.scalar.activation(atn2, q2, AF.Arctan)
                    sgn = ws(6)
                    nc.scalar.activation(sgn, yr, AF.Sign)
                    fx = ws(7)
                    ts(out=fx, in0=xr, scalar1=0.0, scalar2=PI,
                       op0=AL.is_lt, op1=AL.mult)
                    tt(out=fx, in0=fx, in1=sgn, op=AL.mult)
                    tt(out=fx, in0=fx, in1=atn2, op=AL.add)        # phi in (-pi, pi]
                    xs = ws(5)
                    # xs = -KY*phi + 1024.5
                    nc.scalar.activation(xs, fx, AF.Copy, bias=W / 2 + 0.5, scale=-KY)

                    # ---- fracs, table index, weights ----
                    # floor via int round-trip (rounding-mode agnostic):
                    # e = x - f32(int(x)) in (-1,1); L = [e<0]; frac = e+L;
                    # floor = f32(int(x)) - L
                    yi = wk.tile([P, T], I32, tag="icnv")
                    nc.vector.tensor_copy(yi[:], ys)
                    yfb, ey, Ly, wy = ws(0), ws(1), ws(2), ws(3)
                    nc.vector.tensor_copy(yfb, yi[:])
                    tt(out=ey, in0=ys, in1=yfb, op=AL.subtract)
                    ts(out=Ly, in0=ey, scalar1=0.0, scalar2=None, op0=AL.is_lt)
                    tt(out=wy, in0=ey, in1=Ly, op=AL.add)
                    xi = wk.tile([P, T], I32, tag="icnv")
                    nc.vector.tensor_copy(xi[:], xs)
                    xfb, ex, Lx, wx = ws(6), ws(7), ws(8), ws(9)
                    nc.vector.tensor_copy(xfb, xi[:])
                    tt(out=ex, in0=xs, in1=xfb, op=AL.subtract)
                    ts(out=Lx, in0=ex, scalar1=0.0, scalar2=None, op0=AL.is_lt)
                    tt(out=wx, in0=ex, in1=Lx, op=AL.add)
                    fi = ws(10)
                    ts(out=fi, in0=yfb, scalar1=float(TC), scalar2=None, op0=AL.mult)
                    tt(out=fi, in0=fi, in1=xfb, op=AL.add)
                    stt(out=fi, in0=Ly, scalar=float(-TC), in1=fi,
                        op0=AL.mult, op1=AL.add)
                    tt(out=fi, in0=fi, in1=Lx, op=AL.subtract)
                    ts(out=fi, in0=fi, scalar1=0.0, scalar2=float(TR * TC - 1),
                       op0=AL.max, op1=AL.min)
                    oint = wk.tile([P, T], I32, tag="oint")
                    nc.vector.tensor_copy(oint[:], fi)

                    # corner weights: w00 (topL), w01 (botL), w10 (topR), w11 (botR)
                    wgt = wk.tile([P, 4 * T], F32, tag="wgt")
                    u, v = ws(7), ws(11)     # u = 1-wx, v = 1-wy  (ex no longer needed)
                    ts(out=u, in0=wx, scalar1=-1.0, scalar2=1.0, op0=AL.mult, op1=AL.add)
                    ts(out=v, in0=wy, scalar1=-1.0, scalar2=1.0, op0=AL.mult, op1=AL.add)
                    tt(out=wgt[:, 0 * T:1 * T], in0=v, in1=u, op=AL.mult)
                    tt(out=wgt[:, 1 * T:2 * T], in0=wy, in1=u, op=AL.mult)
                    tt(out=wgt[:, 2 * T:3 * T], in0=v, in1=wx, op=AL.mult)
                    tt(out=wgt[:, 3 * T:4 * T], in0=wy, in1=wx, op=AL.mult)

                    # ---- gather: one 48B record per point ----
                    g = gp.tile([P, T * NRECD], F32, tag="g")
                    for j in range(T):
                        nc.gpsimd.indirect_dma_start(
                            out=g[:, j * NRECD:(j + 1) * NRECD],
                            out_offset=None,
                            in_=tbl[:],
                            in_offset=bass.IndirectOffsetOnAxis(
                                ap=oint[:, j:j + 1], axis=0),
                        )

                    # ---- blend ----
                    g3 = g[:].rearrange("p (t d) -> p t d", d=NRECD)
                    smp = bl.tile([P, 3 * T], F32, tag="smp")
                    s3 = smp[:].rearrange("p (t c) -> p t c", c=3)
                    prod = bl.tile([P, 3 * T], F32, tag="prod")
                    p3 = prod[:].rearrange("p (t c) -> p t c", c=3)

                    def wap(i):  # weight i broadcast over c (stride-0 inner dim)
                        a = wgt[:, i * T:(i + 1) * T]
                        return bass.AP(a.tensor, a.offset, a.ap + [(0, 3)])

                    tt(out=s3, in0=g3[:, :, 0:3], in1=wap(0), op=AL.mult)
                    tt(out=p3, in0=g3[:, :, 3:6], in1=wap(1), op=AL.mult)
                    tt(out=s3, in0=s3, in1=p3, op=AL.add)
                    tt(out=p3, in0=g3[:, :, 6:9], in1=wap(2), op=AL.mult)
                    tt(out=s3, in0=s3, in1=p3, op=AL.add)
                    tt(out=p3, in0=g3[:, :, 9:12], in1=wap(3), op=AL.mult)
                    tt(out=s3, in0=s3, in1=p3, op=AL.add)

                    # ---- mask, distance, accumulate ----
                    smp2 = smp[:].rearrange("p (t c) -> p t c", c=3)
                    ssum, dd = ws(0), ws(1)
                    tt(out=ssum, in0=smp2[:, :, 0], in1=smp2[:, :, 1], op=AL.add)
                    tt(out=ssum, in0=ssum, in1=smp2[:, :, 2], op=AL.add)
                    tt(out=ssum, in0=ssum, in1=vld_t[:, sl], op=AL.mult)

                    d3 = prod[:].rearrange("p (t c) -> p t c", c=3)
                    r3 = rgb_t[:, sl3].rearrange("p (t c) -> p t c", c=3)
                    tt(out=d3, in0=s3, in1=r3, op=AL.subtract)
                    tt(out=d3, in0=d3, in1=d3, op=AL.mult)
                    tt(out=dd, in0=d3[:, :, 0], in1=d3[:, :, 1], op=AL.add)
                    tt(out=dd, in0=dd, in1=d3[:, :, 2], op=AL.add)
                    sd = ws(2)
                    nc.scalar.activation(sd, dd, AF.Sqrt)

                    mk = ws(3)
                    part = wk.tile([P, 2], F32, tag="part")
                    ts(out=mk, in0=ssum, scalar1=0.0, scalar2=None, op0=AL.is_gt)
                    mk2 = ws(5)
                    stt(out=mk2, in0=mk, scalar=0.0, in1=mk,
                        op0=AL.add, op1=AL.mult, accum_out=part[:, 1:2])
                    pp = ws(4)
                    stt(out=pp, in0=sd, scalar=0.0, in1=mk,
                        op0=AL.add, op1=AL.mult, accum_out=part[:, 0:1])
                    tt(out=acc[:, b:b + 1], in0=acc[:, b:b + 1],
                       in1=part[:, 0:1], op=AL.add)
                    tt(out=acc[:, BPC + b:BPC + b + 1],
                       in0=acc[:, BPC + b:BPC + b + 1], in1=part[:, 1:2], op=AL.add)

            nc.gpsimd.dma_start(out=out[:, :], in_=acc[:])
    nc.finalize()
    return nc


def _prep_shared(xyz, rgb, img):
    """Host-side input prep shared across cores (layouts + pair table)."""
    # pair table with zero borders
    pad = np.zeros((H + 2, W + 3, 3), np.float32)
    pad[1:H + 1, 1:W + 1] = img
    tbl = np.concatenate(
        [pad[:TR, :TC], pad[1:TR + 1, :TC], pad[:TR, 1:TC + 1],
         pad[1:TR + 1, 1:TC + 1]], axis=-1,
    ).reshape(TR * TC, NRECD)
    tbl = np.ascontiguousarray(tbl)

    # point planes: slot (p, j) <- point id j*128 + p
    nslot = P * L
    xyz_f = np.zeros((nslot, 3), np.float32)
    xyz_f[:N] = xyz[0]
    rgb_f = np.zeros((nslot, 3), np.float32)
    rgb_f[:N] = rgb[0]
    vld_f = np.zeros(nslot, np.float32)
    vld_f[:N] = 1.0

    xyz_pl = xyz_f.reshape(L, P, 3).transpose(1, 2, 0)      # [P, 3, L]
    xyzp = np.ascontiguousarray(xyz_pl).reshape(P, 3 * L)
    rgb_pl = rgb_f.reshape(L, P, 3).transpose(1, 0, 2)      # [P, L, 3] interleaved
    rgbp = np.ascontiguousarray(rgb_pl).reshape(P, 3 * L)
    vldp = np.ascontiguousarray(vld_f.reshape(L, P).T)
    return tbl, xyzp, rgbp, vldp


def kernel(xyz, rgb, img, translation, yaw, pitch, roll):
    xyz = np.asarray(xyz, np.float32)
    rgb = np.asarray(rgb, np.float32)
    img = np.asarray(img, np.float32)
    translation = np.asarray(translation, np.float32)
    yaw = np.asarray(yaw, np.float32)
    pitch = np.asarray(pitch, np.float32)
    roll = np.asarray(roll, np.float32)

    if "nc" not in _CACHED:
        _CACHED["nc"] = _build_bass()
    nc = _CACHED["nc"]

    tbl, xyzp, rgbp, vldp = _prep_shared(xyz, rgb, img)

    in_maps = []
    for c in range(NCORES):
        bs = slice(c * BPC, (c + 1) * BPC)
        posev = np.concatenate([
            yaw[bs, 0], pitch[bs, 0], roll[bs, 0],
            translation[bs, 0, 0], translation[bs, 1, 0], translation[bs, 2, 0],
        ]).astype(np.float32)                              # [6*BPC]
        pose = np.broadcast_to(posev, (P, 6 * BPC)).copy()
        in_maps.append({"tbl": tbl, "xyzp": xyzp, "rgbp": rgbp,
                        "vldp": vldp, "pose": pose})

    res = run_bass_kernel_spmd(nc, in_maps, list(range(NCORES)))

    losses = np.zeros(B, np.float32)
    for c in range(NCORES):
        o = res.results[c]["out"]                          # [P, 2*BPC]
        sums = o.sum(axis=0)                               # [2*BPC]
        for b in range(BPC):
            losses[c * BPC + b] = sums[b] / sums[BPC + b]
    return np.float32(losses.sum()), losses
